# revision 17
# baseline (speedup 1.0000x reference)
"""Trainium2 Bass kernel for nn_MetaAttentionModule (moe_routing).

Computes, for each batch row b:
    scores[b, n]  = expert[b, n, :] . v_e   (+ task.v_t, which is constant
                    across n and therefore cancels exactly in the softmax)
    w[b, :]       = softmax(scores[b, :])
    agg[b, :]     = sum_n w[b, n] * expert[b, n, :]

Sharding: pure data parallel over the batch dim across 8 NeuronCores
(512 rows per core), v replicated. Per core the 512 rows are processed
as 4 chunks of 128 rows (the SBUF partition dim); each chunk is loaded
as 4 quarter-chunks of 4 experts for fine DMA/compute pipelining.

Per-chunk engine assignment:
  - DMA (HWDGE/SP ring): 4x 1MB expert quarter loads;
        (HWDGE/ACT ring): output stores.
  - DVE:  one broadcast multiply expert_q * v_e per quarter (stride-0
          broadcast over the expert dim); a 1-expert X-reduce per quarter;
          softmax small ops.
  - ACT:  3 score reductions per quarter via activation(Copy, accum_out);
          exp (+sum accumulation); PSUM->SBUF copy of the aggregate.
  - GPSIMD: the 16 diag(w_n) builds (identity row-scaled).
  - PE:   warm-up matmuls during the lead-in, then 16 accumulating fp32
          matmuls psum += diag(w_n) @ expert_n per chunk (contraction over
          the 128 batch rows on partitions).
"""

import numpy as np

B, N, E, T = 4096, 16, 512, 128
NCORES = 8
ROWS = B // NCORES  # rows per core
P = 128  # partition chunk (batch rows per chunk)
NCHUNKS = ROWS // P
NQ = 4  # experts per quarter-chunk
QPC = N // NQ  # quarters per chunk

_CACHE = {}


def _build_bass(xin_bufs=12, psum_bufs=3, kh=1, diag_eng="gpsimd", n_warm=7, reps=1, k_off=3, pool_mul_q=(), do_mm=True, do_scores=True):
    import concourse.bass as bass
    import concourse.mybir as mybir
    import concourse.tile as tile
    from concourse import bacc
    from concourse.masks import make_identity

    fp32 = mybir.dt.float32
    Alu = mybir.AluOpType
    Act = mybir.ActivationFunctionType

    nc = bacc.Bacc(
        "TRN2",
        target_bir_lowering=False,
        debug=False,
        enable_asserts=False,
        num_devices=NCORES,
    )
    expert = nc.dram_tensor("expert", (ROWS, N, E), fp32, kind="ExternalInput").ap()
    v = nc.dram_tensor("v", (E + T, 1), fp32, kind="ExternalInput").ap()
    agg = nc.dram_tensor("agg", (ROWS, E), fp32, kind="ExternalOutput").ap()
    attw = nc.dram_tensor("attw", (ROWS, N), fp32, kind="ExternalOutput").ap()

    with tile.TileContext(nc) as tc:
        with (
            tc.tile_pool(name="const", bufs=1) as const_pool,
            tc.tile_pool(name="xin", bufs=xin_bufs) as xin_pool,
            tc.tile_pool(name="small", bufs=3) as small_pool,
            tc.tile_pool(name="prod", bufs=3) as prod_pool,
            tc.tile_pool(name="diag", bufs=8) as diag_pool,
            tc.tile_pool(name="aout", bufs=3) as aout_pool,
            tc.tile_pool(name="psum", bufs=psum_bufs, space="PSUM") as psum_pool,
            tc.tile_pool(name="psum_warm", bufs=1, space="PSUM") as psum_warm_pool,
        ):
            ident = const_pool.tile([P, P], fp32)
            make_identity(nc, ident)
            # v_e broadcast to all partitions: [P, E]
            v_e_b = const_pool.tile([P, E], fp32)
            nc.gpsimd.dma_start(
                out=v_e_b, in_=bass.AP(tensor=v.tensor, offset=0, ap=[[0, P], [1, E]])
            )

            # PE warm-up: harmless matmuls into a scratch PSUM bank while
            # the first chunk's scores are still in flight. Keeps the PE
            # p-state ramp (HAM) out of the first real matmul burst. They
            # read v_e_b, which delays them just enough that they run
            # back-to-back into the first real matmul burst.
            warm_ps = psum_warm_pool.tile([P, E], fp32)
            for _ in range(n_warm):
                nc.tensor.matmul(
                    out=warm_ps,
                    lhsT=ident,
                    rhs=v_e_b,
                    start=True,
                    stop=True,
                )

            import contextlib

            rep_ctx = tc.For_i(0, reps, 1) if reps > 1 else contextlib.nullcontext()
            with rep_ctx:
              for c in range(NCHUNKS):
                r0 = c * P
                quarters = []
                scores = small_pool.tile([P, N], fp32, tag="scores")
                for q in range(QPC):
                    xq = xin_pool.tile([P, NQ, E], fp32, tag="x")
                    nc.sync.dma_start(
                        out=xq, in_=expert[r0 : r0 + P, q * NQ : (q + 1) * NQ, :]
                    )
                    quarters.append(xq)

                    # scores[p, n] = sum_e expert[p,n,e] * v_e[e]
                    prod = prod_pool.tile([P, NQ, E], fp32, tag="prod")
                    v_bcast = bass.AP(
                        tensor=v_e_b.tensor,
                        offset=v_e_b.offset,
                        ap=[v_e_b.ap[0], [0, NQ], [1, E]],
                    )
                    if not do_scores:
                        continue
                    mul_eng = nc.gpsimd if q in pool_mul_q else nc.vector
                    mul_eng.tensor_tensor(
                        out=prod, in0=xq, in1=v_bcast, op=Alu.mult
                    )
                    # first kh experts of the quarter reduced on DVE in one
                    # op, the rest individually on ACT
                    n0 = q * NQ
                    khq = kh[q] if isinstance(kh, (tuple, list)) else kh
                    if khq:
                        nc.vector.tensor_reduce(
                            out=scores[:, n0 : n0 + khq],
                            in_=prod[:, 0:khq, :],
                            axis=mybir.AxisListType.X,
                            op=Alu.add,
                        )
                    for nl in range(khq, NQ):
                        nc.scalar.activation(
                            out=prod[:, nl, :],
                            in_=prod[:, nl, :],
                            func=Act.Copy,
                            bias=0.0,
                            scale=1.0,
                            accum_out=scores[:, n0 + nl : n0 + nl + 1],
                        )

                if not do_scores:
                    nc.vector.memset(scores, 0.0)
                # softmax over n (free dim)
                negmax = small_pool.tile([P, 1], fp32, tag="negmax")
                nc.vector.tensor_reduce(
                    out=negmax,
                    in_=scores,
                    axis=mybir.AxisListType.X,
                    op=Alu.max,
                    negate=True,
                )
                expw = small_pool.tile([P, N], fp32, tag="expw")
                sumexp = small_pool.tile([P, 1], fp32, tag="sumexp")
                nc.scalar.activation(
                    out=expw,
                    in_=scores,
                    func=Act.Exp,
                    bias=negmax[:, 0:1],
                    scale=1.0,
                    accum_out=sumexp,
                )
                rcp = small_pool.tile([P, 1], fp32, tag="rcp")
                nc.vector.reciprocal(out=rcp, in_=sumexp)
                attw_t = small_pool.tile([P, N], fp32, tag="attw")
                nc.vector.tensor_scalar_mul(out=attw_t, in0=expw, scalar1=rcp[:, 0:1])
                nc.scalar.dma_start(out=attw[r0 : r0 + P], in_=attw_t)

                # aggregation: psum += diag(w_n) @ expert_n on PE for the
                # first N-k_off experts; the last k_off via ACT multiply +
                # DVE merge (keeps PE off the critical path at the margin).
                n_pe = (N - k_off) if do_mm else 1
                psum_t = psum_pool.tile([P, E], fp32)
                for n in range(n_pe):
                    dg = diag_pool.tile([P, P], fp32, tag="diag")
                    getattr(nc, diag_eng).tensor_scalar_mul(
                        out=dg, in0=ident, scalar1=attw_t[:, n : n + 1]
                    )
                    nc.tensor.matmul(
                        out=psum_t,
                        lhsT=dg,
                        rhs=quarters[n // NQ][:, n % NQ, :],
                        start=(n == 0),
                        stop=(n == n_pe - 1),
                    )

                agg_sb = aout_pool.tile([P, E], fp32, tag="agg")
                if k_off == 0:
                    nc.scalar.copy(out=agg_sb, in_=psum_t)
                else:
                    tmps = []
                    for n in range(n_pe, N):
                        tmp = aout_pool.tile([P, E], fp32, tag="offtmp")
                        nc.scalar.activation(
                            out=tmp,
                            in_=quarters[n // NQ][:, n % NQ, :],
                            func=Act.Copy,
                            bias=0.0,
                            scale=attw_t[:, n : n + 1],
                        )
                        tmps.append(tmp)
                    acc = tmps[0]
                    for tmp in tmps[1:]:
                        acc2 = aout_pool.tile([P, E], fp32, tag="offacc")
                        nc.vector.tensor_add(acc2, acc, tmp)
                        acc = acc2
                    nc.vector.tensor_add(agg_sb, psum_t, acc)
                nc.scalar.dma_start(out=agg[r0 : r0 + P], in_=agg_sb)

    nc.compile()
    return nc


def _get_nc():
    if "nc" not in _CACHE:
        _CACHE["nc"] = _build_bass()
    return _CACHE["nc"]


def _make_in_maps(expert_np, v_np):
    return [
        {
            "expert": expert_np[c * ROWS : (c + 1) * ROWS],
            "v": v_np,
        }
        for c in range(NCORES)
    ]


def run(expert_np, v_np, **kwargs):
    """Run on 8 cores; returns (BassKernelResults, agg, attw)."""
    from concourse.bass_utils import run_bass_kernel_spmd

    nc = _get_nc()
    res = run_bass_kernel_spmd(
        nc, _make_in_maps(expert_np, v_np), core_ids=list(range(NCORES)),
        **kwargs,
    )
    agg = np.concatenate([r["agg"] for r in res.results], axis=0)
    attw = np.concatenate([r["attw"] for r in res.results], axis=0)
    return res, agg, attw


def kernel(scene_repr=None, task_anchor=None, expert_reprs=None, v=None):
    expert_np = np.ascontiguousarray(np.asarray(expert_reprs, dtype=np.float32))
    v_np = np.ascontiguousarray(np.asarray(v, dtype=np.float32))
    _, agg, attw = run(expert_np, v_np)
    return agg, attw
